# revision 8
# baseline (speedup 1.0000x reference)
"""Trainium2 Bass kernel for nn_FEM_35072702939287 (attention + BN + channel gate).

Math restructuring (validated vs reference):
  A[t,s] = (Wk x + bk)[:,t] . (Wq x + bq)[:,s]; softmax over s drops
  t-row constants, so with G = Wk^T Wq, r = Wq^T bk:
      A^T[s,t] = (G X)[:,s] . X[:,t]  +  (r^T X)[s]
  The (r^T X)[s] term is constant along t == a per-partition bias, so it
  is folded into exp's free bias (together with a range-shift EXPB that
  cancels in the softmax ratio).  This makes the A^T matmul contraction
  exactly K=64, so two A^T matmuls run CONCURRENTLY in the PE array on
  row strips 0-63 / 64-127 (lhsT/rhs duplicated across both strips).
  The PV accumulation packs its two 33-row output groups at partition
  offsets 0/64 of one PSUM tile (column strips), so PV matmul pairs also
  run concurrently (the accumulator banks are pre-cleared by a zero
  matmul because has_written is cleared bank-wide).  D = sum_s exp falls
  out of the PV matmul via a ones column in V^T; division by D is folded
  past the Wt conv, whose bias bt cancels under BN.  BN batch stats are
  all-reduced across the 8 cores (a warmup collective runs at kernel
  start to absorb one-time CC setup).

Sharding: data-parallel over batch N=16 -> 2 batches per core x 8 cores.
"""

import numpy as np

N_CORES = 8
N, C, T, V = 16, 64, 64, 25
TV = T * V            # 1600
IC = 32
NB = N // N_CORES     # batches per core
EPS = 1e-5
NSB = 13              # 12 full 128-row s-blocks + one 64-row tail
SB = [(j * 128, 128) for j in range(12)] + [(1536, 64)]
HALVES = [(0, 800), (800, 1600)]
CH_H = [(0, 512), (512, 800)]   # psum-bank-aligned chunks inside an 800 half
EXPB = -4.0                      # free exp bias (cancels in softmax)


def _build(nc):
    import concourse.tile as tile
    from concourse import mybir
    from contextlib import ExitStack

    f32 = mybir.dt.float32
    f16 = mybir.dt.float16
    AF = mybir.ActivationFunctionType
    ALU = mybir.AluOpType
    AX = mybir.AxisListType

    # ---------------- DRAM I/O (weights pre-transposed on host) ------
    x_in = nc.dram_tensor("x_in", [NB, C, TV], f32, kind="ExternalInput").ap()
    wq_d = nc.dram_tensor("wq", [IC, C], f32, kind="ExternalInput").ap()
    wk_d = nc.dram_tensor("wk", [IC, C], f32, kind="ExternalInput").ap()
    bk_d = nc.dram_tensor("bk", [IC, 1], f32, kind="ExternalInput").ap()
    wv_d = nc.dram_tensor("wv", [C, IC], f32, kind="ExternalInput").ap()
    bv_d = nc.dram_tensor("bv", [1, IC], f32, kind="ExternalInput").ap()
    wt_d = nc.dram_tensor("wt", [IC, C], f32, kind="ExternalInput").ap()
    bt_d = nc.dram_tensor("bt", [1, C], f32, kind="ExternalInput").ap()
    gm_d = nc.dram_tensor("gamma", [C, 1], f32, kind="ExternalInput").ap()
    bt2_d = nc.dram_tensor("beta", [C, 1], f32, kind="ExternalInput").ap()
    w1_d = nc.dram_tensor("w1", [C, C // 16], f32, kind="ExternalInput").ap()
    b1_d = nc.dram_tensor("b1", [C // 16, 1], f32, kind="ExternalInput").ap()
    w2_d = nc.dram_tensor("w2", [C // 16, C], f32, kind="ExternalInput").ap()
    b2_d = nc.dram_tensor("b2", [C, 1], f32, kind="ExternalInput").ap()
    out_d = nc.dram_tensor("out", [NB, C, TV], f32, kind="ExternalOutput").ap()

    R = C // 16  # 4

    with tile.TileContext(nc) as tc, ExitStack() as ctx:
        consts = ctx.enter_context(tc.tile_pool(name="consts", bufs=1))
        xpool = ctx.enter_context(tc.tile_pool(name="xpool", bufs=2))
        workp = ctx.enter_context(tc.tile_pool(name="workp", bufs=2))
        statp = ctx.enter_context(tc.tile_pool(name="statp", bufs=1))
        # psA: A^T psums + all transient psums (H, V^T, gate) -> 4 banks
        psA = ctx.enter_context(tc.tile_pool(name="psA", bufs=2, space="PSUM"))
        # psB: stacked PV accumulator + remainder psums -> 4 banks
        psB = ctx.enter_context(tc.tile_pool(name="psB", bufs=2, space="PSUM"))
        dramp = ctx.enter_context(tc.tile_pool(name="dramp", bufs=1, space="DRAM"))

        # ---------------- input DMAs (sync queue) -----------------------
        xa = [None] * NB      # [65, TV] f32 : [X; 1]
        for b in range(NB):
            t = xpool.tile([C + 1, TV], f32, name="xa", tag="xa")
            xa[b] = t
            nc.sync.dma_start(out=t[0:C, :], in_=x_in[b])
            nc.vector.memset(t[C:C + 1, :], 1.0)

        # ---------------- warmup collective (absorbs CC setup) ----------
        z2 = consts.tile([C, 2], f32)
        nc.vector.memset(z2, 0.0)
        wu_in = dramp.tile([C, 2], f32, name="wu_in")
        wu_out = dramp.tile([C, 2], f32, name="wu_out", addr_space="Shared")
        nc.sync.dma_start(out=wu_in, in_=z2)
        nc.gpsimd.collective_compute(
            "AllReduce", ALU.add, ins=[wu_in.opt()], outs=[wu_out.opt()],
            replica_groups=[list(range(N_CORES))],
        )

        # ---------------- constants / weights ---------------------------
        onesf = consts.tile([128, C], f32)
        nc.vector.memset(onesf, 1.0)
        ones128 = consts.tile([128, C], f16)
        nc.vector.tensor_copy(ones128, onesf)
        zcol = consts.tile([1, 128], f16)
        nc.vector.memset(zcol, 0.0)
        zrhs = consts.tile([1, 512], f16)
        nc.vector.memset(zrhs, 0.0)
        # warm up the ACT table early so exp's table set loads once
        warmz = consts.tile([1, 1], f32)
        nc.vector.memset(warmz, 1.0)
        warmo = consts.tile([1, 1], f32)
        nc.scalar.activation(warmo, warmz, AF.Exp)

        wq_sb = consts.tile([IC, C], f32)
        nc.scalar.dma_start(out=wq_sb, in_=wq_d)
        wkbk = consts.tile([IC, C + 2], f32)
        nc.vector.memset(wkbk[:, C + 1:C + 2], 0.0)
        nc.scalar.dma_start(out=wkbk[:, 0:C], in_=wk_d)
        nc.scalar.dma_start(out=wkbk[:, C:C + 1], in_=bk_d)

        # V^T weights, padded to 34 cols (even moving size)
        wvt_aug = consts.tile([C + 1, IC + 2], f32)
        nc.vector.memset(wvt_aug, 0.0)
        nc.gpsimd.dma_start(out=wvt_aug[0:C, 0:IC], in_=wv_d)
        nc.gpsimd.dma_start(out=wvt_aug[C:C + 1, 0:IC], in_=bv_d)
        nc.vector.memset(wvt_aug[C:C + 1, IC:IC + 1], 1.0)
        wvt_r = consts.tile([C + 1, IC + 2], f16)
        nc.vector.tensor_copy(wvt_r, wvt_aug)

        # Wt^T f16, duplicated at partitions 0-31 / 64-95 (bt drops under BN)
        wt_rep = consts.tile([IC, C], f32)
        nc.gpsimd.dma_start(out=wt_rep, in_=wt_d)
        wt128 = consts.tile([128, C], f16)
        nc.vector.tensor_copy(wt128[0:IC, :], wt_rep)
        nc.sync.dma_start(out=wt128[64:64 + IC, :], in_=wt128[0:IC, :])

        w1t = consts.tile([C, R], f32)
        nc.gpsimd.dma_start(out=w1t, in_=w1_d)
        w2t = consts.tile([R, C], f32)
        nc.gpsimd.dma_start(out=w2t, in_=w2_d)
        b1_sb = consts.tile([R, 1], f32)
        nc.gpsimd.dma_start(out=b1_sb, in_=b1_d)
        b2_sb = consts.tile([C, 1], f32)
        nc.gpsimd.dma_start(out=b2_sb, in_=b2_d)
        b2n = consts.tile([C, 1], f32)
        nc.vector.tensor_scalar_mul(b2n, b2_sb, -1.0)
        gamma_sb = consts.tile([C, 1], f32)
        nc.gpsimd.dma_start(out=gamma_sb, in_=gm_d)
        beta_sb = consts.tile([C, 1], f32)
        nc.gpsimd.dma_start(out=beta_sb, in_=bt2_d)

        # G^T | r  =  Wq^T @ [Wk | bk];  gr2 = [G^T G^T] for 128-row H
        psg = psA.tile([C, C + 2], f32, name="psg", tag="a")
        nc.tensor.matmul(psg, lhsT=wq_sb, rhs=wkbk, start=True, stop=True)
        gr2 = consts.tile([C, 128], f16)
        nc.vector.tensor_copy(gr2[:, 0:C], psg[:, 0:C])
        nc.vector.tensor_copy(gr2[:, C:128], psg[:, 0:C])
        rv16 = consts.tile([C, 2], f16)
        nc.vector.memset(rv16[:, 1:2], 0.0)
        nc.vector.tensor_copy(rv16[:, 0:1], psg[:, C:C + 1])

        # ---------------- per-batch state ----------------
        xr = [None] * NB      # [65, TV] f16 : [X; 1]  (V^T lhsT)
        xr2 = [None] * NB     # [128, TV] f16 : X duplicated on both strips
        ha2 = [None] * NB     # [128, TV] f16 : G X duplicated on both strips
        vt1 = [None] * NB     # [128, 13, 33] f16 : [V^T | 1] per s-block
        eb = [None] * NB      # [128, 13, TV] f16 : exp(A^T + rX + EXPB)
        p2 = [None] * NB      # [64, TV] f32 : p2 = (Wt p) / D  (pre-BN)
        rxc = [None] * NB     # [128, NSB] f32 : (r^T X)[s] + EXPB per block
        avgs = statp.tile([C, NB], f32)
        stats = statp.tile([C, NB * 4, 6], f32)

        def prologue(b):
            t = xa[b]
            tr = xpool.tile([C + 1, TV], f16, name="xr", tag="xr")
            xr[b] = tr
            nc.vector.tensor_copy(tr, t)
            t2 = xpool.tile([128, TV], f16, name="xr2", tag="xr2")
            xr2[b] = t2
            nc.vector.tensor_copy(t2[0:C, :], tr[0:C, :])
            nc.sync.dma_start(out=t2[C:128, :], in_=t2[0:C, :])
            h2 = xpool.tile([128, TV], f16, name="ha2", tag="ha2")
            ha2[b] = h2
            for (h0, h1) in HALVES:
                hps = psA.tile([128, 800], f32, name="hps", tag="a")
                for (c0, c1) in CH_H:
                    nc.tensor.matmul(hps[:, c0:c1], lhsT=gr2,
                                     rhs=tr[0:C, h0 + c0:h0 + c1],
                                     start=True, stop=True)
                nc.vector.tensor_copy(h2[:, h0:h1], hps)
            vt1[b] = xpool.tile([128, NSB, IC + 1], f16, name="vt1", tag="vt1")
            eb[b] = xpool.tile([128, NSB, TV], f16, name="eb", tag="eb")
            p2[b] = xpool.tile([C, TV], f32, name="p2", tag="p2")
            rxc[b] = xpool.tile([128, NSB], f32, name="rxc", tag="rxc")
            vps = psA.tile([128, NSB, IC + 2], f32, name="vps", tag="a")
            for j, (off, p) in enumerate(SB):
                nc.tensor.matmul(vps[0:p, j, :], lhsT=tr[:, off:off + p],
                                 rhs=wvt_r, start=True, stop=True)
            nc.vector.tensor_copy(vt1[b], vps[:, :, 0:IC + 1])
            # rX per s-block, transposed into partitions via tiny matmuls
            rxp = psA.tile([128, NSB, 2], f32, name="rxp", tag="a")
            for j, (off, p) in enumerate(SB):
                nc.tensor.matmul(rxp[0:p, j, :], lhsT=t2[0:C, off:off + p],
                                 rhs=rv16, start=True, stop=True)
            nc.vector.tensor_scalar_add(rxc[b], rxp[:, :, 0], EXPB)

        def phase1(b):
            """Row-packed A^T pairs -> exp -> col-packed PV pairs.
            PV issue lags A by LAG blocks so a PSUM-buffer wait on the
            (in-order) PE queue never starves the exp pipeline."""
            pacc = psB.tile([128, 800], f32, name="pacc", tag="b")
            nc.tensor.matmul(pacc[:, 0:512], lhsT=zcol, rhs=zrhs[:, 0:512],
                             start=True, stop=False, skip_group_check=True)
            nc.tensor.matmul(pacc[:, 512:800], lhsT=zcol, rhs=zrhs[:, 0:288],
                             start=True, stop=False, skip_group_check=True)
            LAG = 3
            for jj in range(NSB + LAG):
                if jj < NSB:
                    j, (off, p) = jj, SB[jj]
                    aps = [psA.tile([128, 800], f32, name=f"aps{hi}", tag="a")
                           for hi in range(2)]
                    for (c0, c1) in CH_H:
                        for hi, (h0, h1) in enumerate(HALVES):
                            nc.tensor.matmul(
                                aps[hi][0:p, c0:c1],
                                lhsT=ha2[b][C * hi:C * hi + C, off:off + p],
                                rhs=xr2[b][C * hi:C * hi + C, h0 + c0:h0 + c1],
                                start=True, stop=True)
                    for hi, (h0, h1) in enumerate(HALVES):
                        nc.scalar.activation(eb[b][0:p, j, h0:h1],
                                             aps[hi][0:p, :], AF.Exp,
                                             bias=rxc[b][0:p, j:j + 1])
                if jj >= LAG:
                    j, (off, p) = jj - LAG, SB[jj - LAG]
                    for (c0, c1) in CH_H:
                        for ti, (t0, t1) in enumerate(HALVES):
                            nc.tensor.matmul(
                                pacc[C * ti:C * ti + IC + 1, c0:c1],
                                lhsT=vt1[b][0:p, j, :],
                                rhs=eb[b][0:p, j, t0 + c0:t0 + c1],
                                start=False, stop=(j == NSB - 1),
                                skip_group_check=True)
            return pacc

        def remainder(b, pacc):
            """PD -> Wt conv -> /D -> bn_stats per chunk (f16 matmuls,
            row strips 0-63 / 64-127 hold the two t-halves)."""
            pd = workp.tile([128, 800], f16, name="pd", tag="pd")
            nc.vector.tensor_copy(pd, pacc)
            for ti, (t0, t1) in enumerate(HALVES):
                o = C * ti
                for ci, (c0, c1) in enumerate(CH_H):
                    w = c1 - c0
                    p2ps = psB.tile([C, 512], f32, name="p2ps", tag="b")
                    dps = psB.tile([C, 512], f32, name="dps", tag="b")
                    nc.tensor.matmul(p2ps[:, 0:w], lhsT=wt128[o:o + IC, :],
                                     rhs=pd[o:o + IC, c0:c1],
                                     start=True, stop=True)
                    nc.tensor.matmul(dps[:, 0:w],
                                     lhsT=ones128[o + IC:o + IC + 1, :],
                                     rhs=pd[o + IC:o + IC + 1, c0:c1],
                                     start=True, stop=True,
                                     tile_position=(o + IC, 0))
                    rrep = workp.tile([C, 512], f32, name="rrep", tag="rrep")
                    nc.vector.reciprocal_approx_fast(out=rrep[:, 0:w],
                                                     in_=dps[:, 0:w])
                    nc.vector.tensor_mul(p2[b][:, t0 + c0:t0 + c1],
                                         p2ps[:, 0:w], rrep[:, 0:w])
                    nc.vector.bn_stats(stats[:, 4 * b + 2 * ti + ci, :],
                                       p2[b][:, t0 + c0:t0 + c1])

        # prologues for BOTH batches first: keeps phase1(0)+phase1(1)
        # back-to-back with no PE/ACT drain at the batch boundary.
        prologue(0)
        prologue(1)
        pa0 = phase1(0)

        # ---------------- channel gate (PE/DVE slack under phase1) ------
        for b in range(NB):
            nc.vector.reduce_sum(avgs[:, b:b + 1], xa[b][0:C, :], axis=AX.X)
        hps2 = psA.tile([R, NB], f32, name="hps2", tag="a")
        nc.tensor.matmul(hps2, lhsT=w1t, rhs=avgs, start=True, stop=True)
        h_pre = statp.tile([R, NB], f32)
        nc.vector.tensor_scalar(h_pre, hps2, 1.0 / TV, b1_sb,
                                op0=ALU.mult, op1=ALU.add)
        h_sb = statp.tile([R, NB], f32)
        nc.vector.tensor_scalar_max(h_sb, h_pre, 0.0)
        zps = psA.tile([C, NB], f32, name="zps", tag="a")
        nc.tensor.matmul(zps, lhsT=w2t, rhs=h_sb, start=True, stop=True)
        eg = statp.tile([C, NB], f32)
        nc.scalar.activation(eg, zps, AF.Exp, bias=b2n, scale=-1.0)
        gp1 = statp.tile([C, NB], f32)
        nc.vector.tensor_scalar_add(gp1, eg, 1.0)
        gate = statp.tile([C, NB], f32)
        nc.vector.reciprocal(gate, gp1)

        remainder(0, pa0)
        pa1 = phase1(1)
        remainder(1, pa1)

        # ---------------- BN stats: local -> allreduce -> global --------
        mv = statp.tile([C, 2], f32)
        nc.vector.bn_aggr(out=mv, in_=stats)
        m2 = statp.tile([C, 1], f32)
        nc.vector.tensor_mul(m2, mv[:, 0:1], mv[:, 0:1])
        ex2 = statp.tile([C, 1], f32)
        nc.vector.tensor_add(ex2, mv[:, 1:2], m2)
        sums = statp.tile([C, 2], f32)
        cnt_local = float(NB * TV)
        nc.vector.tensor_scalar_mul(sums[:, 0:1], mv[:, 0:1], cnt_local)
        nc.vector.tensor_scalar_mul(sums[:, 1:2], ex2, cnt_local)

        cc_in = dramp.tile([C, 2], f32, name="cc_in")
        cc_out = dramp.tile([C, 2], f32, name="cc_out", addr_space="Shared")
        nc.sync.dma_start(out=cc_in, in_=sums)
        nc.gpsimd.collective_compute(
            "AllReduce",
            ALU.add,
            ins=[cc_in.opt()],
            outs=[cc_out.opt()],
            replica_groups=[list(range(N_CORES))],
        )

        # w_b = gate (.) p2_b overlaps the collective
        wts = [None] * NB
        for b in range(NB):
            u = workp.tile([C, TV], f32, name="u", tag="u")
            wts[b] = u
            nc.vector.tensor_scalar_mul(u, p2[b], gate[:, b:b + 1])

        gs = statp.tile([C, 2], f32)
        nc.sync.dma_start(out=gs, in_=cc_out)

        inv_cnt = 1.0 / (N * TV)
        mean_g = statp.tile([C, 1], f32)
        nc.vector.tensor_scalar_mul(mean_g, gs[:, 0:1], inv_cnt)
        q_g = statp.tile([C, 1], f32)
        nc.vector.tensor_scalar_mul(q_g, gs[:, 1:2], inv_cnt)
        mg2 = statp.tile([C, 1], f32)
        nc.vector.tensor_mul(mg2, mean_g, mean_g)
        var_g = statp.tile([C, 1], f32)
        nc.vector.tensor_sub(var_g, q_g, mg2)
        ve = statp.tile([C, 1], f32)
        nc.vector.tensor_scalar_add(ve, var_g, EPS)
        magic = statp.tile([C, 1], mybir.dt.int32)
        nc.vector.memset(magic, 0x5F3759DF)
        hsh = statp.tile([C, 1], mybir.dt.int32)
        nc.vector.tensor_scalar(hsh, ve.bitcast(mybir.dt.int32), 1, None,
                                op0=ALU.arith_shift_right)
        yi = statp.tile([C, 1], mybir.dt.int32)
        nc.vector.tensor_sub(yi, magic, hsh)
        r1 = statp.tile([C, 1], f32)
        rstd = statp.tile([C, 1], f32)
        t1 = statp.tile([C, 1], f32)
        t3 = statp.tile([C, 1], f32)
        y = yi.bitcast(f32)
        for it, dst in ((0, r1), (1, rstd)):
            nc.vector.tensor_mul(t1, y, y)
            nc.vector.tensor_mul(t1, t1, ve)
            nc.vector.tensor_scalar(t3, t1, -0.5, 1.5, op0=ALU.mult, op1=ALU.add)
            nc.vector.tensor_mul(dst, y, t3)
            y = dst
        sc = statp.tile([C, 1], f32)
        nc.vector.tensor_mul(sc, gamma_sb, rstd)
        msc = statp.tile([C, 1], f32)
        nc.vector.tensor_mul(msc, mean_g, sc)
        sh = statp.tile([C, 1], f32)
        nc.vector.tensor_sub(sh, beta_sb, msc)

        # ------------- finalize: out = sc*(gate*p2) + (x + gate*sh) -----
        for b in range(NB):
            d_b = statp.tile([C, 1], f32, name=f"d_{b}")
            nc.vector.tensor_mul(d_b, gate[:, b:b + 1], sh)
            x3 = workp.tile([C, TV], f32, name="x3", tag="x3")
            nc.scalar.activation(x3, xa[b][0:C, :], AF.Identity, bias=d_b)
            osb = workp.tile([C, TV], f32, name="osb", tag="osb")
            nc.vector.scalar_tensor_tensor(out=osb, in0=wts[b], scalar=sc,
                                           in1=x3, op0=ALU.mult, op1=ALU.add)
            nc.sync.dma_start(out=out_d[b], in_=osb)


_CACHE = {}


def _get_compiled():
    if "nc" in _CACHE:
        return _CACHE["nc"]
    import concourse.bacc as bacc

    nc = bacc.Bacc("TRN2", target_bir_lowering=False, debug=False,
                   enable_asserts=False, num_devices=N_CORES)
    _build(nc)
    nc.compile()
    _CACHE["nc"] = nc
    return nc


def _run(inputs, trace=False, **kw):
    from concourse import bass_utils

    nc = _get_compiled()
    x = np.ascontiguousarray(np.asarray(inputs["x"], dtype=np.float32))
    x = x.reshape(N, C, TV)
    f = lambda a: np.ascontiguousarray(np.asarray(a, dtype=np.float32))
    common = {
        "wq": f(inputs["Wq"]),
        "wk": f(inputs["Wk"]),
        "bk": f(inputs["bk"]).reshape(IC, 1),
        "wv": np.ascontiguousarray(f(inputs["Wv"]).T),
        "bv": f(inputs["bv"]).reshape(1, IC),
        "wt": np.ascontiguousarray(f(inputs["Wt"]).T),
        "bt": f(inputs["bt"]).reshape(1, C),
        "gamma": f(inputs["gamma"]).reshape(C, 1),
        "beta": f(inputs["beta"]).reshape(C, 1),
        "w1": np.ascontiguousarray(f(inputs["W1"]).T),
        "b1": f(inputs["b1"]).reshape(C // 16, 1),
        "w2": np.ascontiguousarray(f(inputs["W2"]).T),
        "b2": f(inputs["b2"]).reshape(C, 1),
    }
    in_maps = []
    for c in range(N_CORES):
        m = dict(common)
        m["x_in"] = np.ascontiguousarray(x[c * NB:(c + 1) * NB])
        in_maps.append(m)
    try:
        res = bass_utils.run_bass_kernel_spmd(
            nc, in_maps, core_ids=list(range(N_CORES)), trace=trace, **kw)
    except Exception:
        import time as _time
        _time.sleep(5)
        res = bass_utils.run_bass_kernel_spmd(
            nc, in_maps, core_ids=list(range(N_CORES)), trace=False, **kw)
    out = np.concatenate([res.results[c]["out"] for c in range(N_CORES)], axis=0)
    return out.reshape(N, C, T, V).astype(np.float32), res


def kernel(**inputs):
    return _run(inputs, trace=False)[0]


# revision 13
# speedup vs baseline: 1.0998x; 1.0998x over previous
"""Trainium2 Bass kernel for nn_FEM_35072702939287 (attention + BN + channel gate).

Math restructuring (validated vs reference):
  A[t,s] = (Wk x + bk)[:,t] . (Wq x + bq)[:,s]; softmax over s drops
  t-row constants, so with G = Wk^T Wq, r = Wq^T bk:
      A^T[s,t] = (G X)[:,s] . X[:,t]  +  (r^T X)[s]
  The (r^T X)[s] term is constant along t == a per-partition bias, so it
  is folded into exp's free bias (together with a range-shift EXPB that
  cancels in the softmax ratio).  This makes the A^T matmul contraction
  exactly K=64, so two A^T matmuls run CONCURRENTLY in the PE array on
  row strips 0-63 / 64-127 (lhsT/rhs duplicated across both strips).
  The PV accumulation packs its two 33-row output groups at partition
  offsets 0/64 of one PSUM tile (column strips), so PV matmul pairs also
  run concurrently (the accumulator banks are pre-cleared by a zero
  matmul because has_written is cleared bank-wide).  D = sum_s exp falls
  out of the PV matmul via a ones column in V^T; division by D is folded
  past the Wt conv, whose bias bt cancels under BN.  BN batch stats are
  all-reduced across the 8 cores (a warmup collective runs at kernel
  start to absorb one-time CC setup).

Sharding: data-parallel over batch N=16 -> 2 batches per core x 8 cores.
"""

import numpy as np

N_CORES = 8
N, C, T, V = 16, 64, 64, 25
TV = T * V            # 1600
IC = 32
NB = N // N_CORES     # batches per core
EPS = 1e-5
NSB = 13              # 12 full 128-row s-blocks + one 64-row tail
SB = [(j * 128, 128) for j in range(12)] + [(1536, 64)]
HALVES = [(0, 800), (800, 1600)]
CH_H = [(0, 512), (512, 800)]   # psum-bank-aligned chunks inside an 800 half
EXPB = -4.0                      # free exp bias (cancels in softmax)


def _build(nc):
    import concourse.tile as tile
    from concourse import mybir
    from contextlib import ExitStack

    f32 = mybir.dt.float32
    f16 = mybir.dt.float16
    AF = mybir.ActivationFunctionType
    ALU = mybir.AluOpType
    AX = mybir.AxisListType

    # ---------------- DRAM I/O (weights pre-transposed on host) ------
    x_in = nc.dram_tensor("x_in", [NB, C, TV], f32, kind="ExternalInput").ap()
    wq_d = nc.dram_tensor("wq", [IC, C], f32, kind="ExternalInput").ap()
    wk_d = nc.dram_tensor("wk", [IC, C], f32, kind="ExternalInput").ap()
    bk_d = nc.dram_tensor("bk", [IC, 1], f32, kind="ExternalInput").ap()
    wv_d = nc.dram_tensor("wv", [C, IC], f32, kind="ExternalInput").ap()
    bv_d = nc.dram_tensor("bv", [1, IC], f32, kind="ExternalInput").ap()
    wt_d = nc.dram_tensor("wt", [IC, C], f32, kind="ExternalInput").ap()
    bt_d = nc.dram_tensor("bt", [1, C], f32, kind="ExternalInput").ap()
    gm_d = nc.dram_tensor("gamma", [C, 1], f32, kind="ExternalInput").ap()
    bt2_d = nc.dram_tensor("beta", [C, 1], f32, kind="ExternalInput").ap()
    w1_d = nc.dram_tensor("w1", [C, C // 16], f32, kind="ExternalInput").ap()
    b1_d = nc.dram_tensor("b1", [C // 16, 1], f32, kind="ExternalInput").ap()
    w2_d = nc.dram_tensor("w2", [C // 16, C], f32, kind="ExternalInput").ap()
    b2_d = nc.dram_tensor("b2", [C, 1], f32, kind="ExternalInput").ap()
    out_d = nc.dram_tensor("out", [NB, C, TV], f32, kind="ExternalOutput").ap()

    R = C // 16  # 4

    with tile.TileContext(nc) as tc, ExitStack() as ctx:
        consts = ctx.enter_context(tc.tile_pool(name="consts", bufs=1))
        xpool = ctx.enter_context(tc.tile_pool(name="xpool", bufs=2))
        workp = ctx.enter_context(tc.tile_pool(name="workp", bufs=2))
        statp = ctx.enter_context(tc.tile_pool(name="statp", bufs=1))
        # psA: A^T psums + all transient psums (H, V^T, gate) -> 4 banks
        psA = ctx.enter_context(tc.tile_pool(name="psA", bufs=2, space="PSUM"))
        # psB: stacked PV accumulator + remainder psums -> 4 banks
        psB = ctx.enter_context(tc.tile_pool(name="psB", bufs=2, space="PSUM"))
        dramp = ctx.enter_context(tc.tile_pool(name="dramp", bufs=1, space="DRAM"))

        # ---------------- input DMAs (sync queue) -----------------------
        xa = [None] * NB      # [65, TV] f32 : [X; 1]
        for b in range(NB):
            t = xpool.tile([C + 1, TV], f32, name="xa", tag="xa")
            xa[b] = t
            nc.sync.dma_start(out=t[0:C, :], in_=x_in[b])
            nc.vector.memset(t[C:C + 1, :], 1.0)

        # ---------------- warmup collective (absorbs CC setup) ----------
        z2 = consts.tile([C, 2], f32)
        nc.vector.memset(z2, 0.0)
        wu_in = dramp.tile([C, 2], f32, name="wu_in")
        wu_out = dramp.tile([C, 2], f32, name="wu_out", addr_space="Shared")
        nc.sync.dma_start(out=wu_in, in_=z2)
        nc.gpsimd.collective_compute(
            "AllReduce", ALU.add, ins=[wu_in.opt()], outs=[wu_out.opt()],
            replica_groups=[list(range(N_CORES))],
        )

        # ---------------- constants / weights ---------------------------
        onesf = consts.tile([128, C], f32)
        nc.vector.memset(onesf, 1.0)
        ones128 = consts.tile([128, C], f16)
        nc.vector.tensor_copy(ones128, onesf)
        zcol = consts.tile([1, 128], f16)
        nc.vector.memset(zcol, 0.0)
        zrhs = consts.tile([1, 512], f16)
        nc.vector.memset(zrhs, 0.0)
        # warm up the ACT table early so exp's table set loads once
        warmz = consts.tile([1, 1], f32)
        nc.vector.memset(warmz, 1.0)
        warmo = consts.tile([1, 1], f32)
        nc.scalar.activation(warmo, warmz, AF.Exp)

        wq_sb = consts.tile([IC, C], f32)
        nc.scalar.dma_start(out=wq_sb, in_=wq_d)
        wkbk = consts.tile([IC, C + 2], f32)
        nc.vector.memset(wkbk[:, C + 1:C + 2], 0.0)
        nc.scalar.dma_start(out=wkbk[:, 0:C], in_=wk_d)
        nc.scalar.dma_start(out=wkbk[:, C:C + 1], in_=bk_d)

        # V^T weights, padded to 34 cols (even moving size)
        wvt_aug = consts.tile([C + 1, IC + 2], f32)
        nc.vector.memset(wvt_aug, 0.0)
        nc.gpsimd.dma_start(out=wvt_aug[0:C, 0:IC], in_=wv_d)
        nc.gpsimd.dma_start(out=wvt_aug[C:C + 1, 0:IC], in_=bv_d)
        nc.vector.memset(wvt_aug[C:C + 1, IC:IC + 1], 1.0)
        wvt_r = consts.tile([C + 1, IC + 2], f16)
        nc.vector.tensor_copy(wvt_r, wvt_aug)

        # Wt^T f16, duplicated at partitions 0-31 / 64-95 (bt drops under BN)
        wt_rep = consts.tile([IC, C], f32)
        nc.gpsimd.dma_start(out=wt_rep, in_=wt_d)
        wt128 = consts.tile([128, C], f16)
        nc.vector.tensor_copy(wt128[0:IC, :], wt_rep)
        nc.sync.dma_start(out=wt128[64:64 + IC, :], in_=wt128[0:IC, :])

        w1t = consts.tile([C, R], f32)
        nc.gpsimd.dma_start(out=w1t, in_=w1_d)
        w2t = consts.tile([R, C], f32)
        nc.gpsimd.dma_start(out=w2t, in_=w2_d)
        b1_sb = consts.tile([R, 1], f32)
        nc.gpsimd.dma_start(out=b1_sb, in_=b1_d)
        b2_sb = consts.tile([C, 1], f32)
        nc.gpsimd.dma_start(out=b2_sb, in_=b2_d)
        b2n = consts.tile([C, 1], f32)
        nc.vector.tensor_scalar_mul(b2n, b2_sb, -1.0)
        gamma_sb = consts.tile([C, 1], f32)
        nc.gpsimd.dma_start(out=gamma_sb, in_=gm_d)
        beta_sb = consts.tile([C, 1], f32)
        nc.gpsimd.dma_start(out=beta_sb, in_=bt2_d)

        # G^T | r  =  Wq^T @ [Wk | bk];  gr2 = [G^T G^T] for 128-row H
        psg = psA.tile([C, C + 2], f32, name="psg", tag="a")
        nc.tensor.matmul(psg, lhsT=wq_sb, rhs=wkbk, start=True, stop=True)
        gr2 = consts.tile([C, 128], f16)
        nc.vector.tensor_copy(gr2[:, 0:C], psg[:, 0:C])
        nc.vector.tensor_copy(gr2[:, C:128], psg[:, 0:C])
        rv16 = consts.tile([C, 2], f16)
        nc.vector.memset(rv16[:, 1:2], 0.0)
        nc.vector.tensor_copy(rv16[:, 0:1], psg[:, C:C + 1])

        # ---------------- per-batch state ----------------
        xr = [None] * NB      # [65, TV] f16 : [X; 1]  (V^T lhsT)
        xr2 = [None] * NB     # [128, TV] f16 : X duplicated on both strips
        ha2 = [None] * NB     # [128, TV] f16 : G X duplicated on both strips
        vt1 = [None] * NB     # [128, 13, 33] f16 : [V^T | 1] per s-block
        eb = [None] * NB      # [128, 13, TV] f16 : exp(A^T + rX + EXPB)
        p2 = [None] * NB      # [64, TV] f32 : p2 = (Wt p) / D  (pre-BN)
        rxc = [None] * NB     # [128, NSB] f32 : (r^T X)[s] + EXPB per block
        avgs = statp.tile([C, NB], f32)
        stats = statp.tile([C, NB * 4, 6], f32)

        def prologue(b):
            t = xa[b]
            tr = xpool.tile([C + 1, TV], f16, name="xr", tag="xr")
            xr[b] = tr
            nc.vector.tensor_copy(tr, t)
            t2 = xpool.tile([128, TV], f16, name="xr2", tag="xr2")
            xr2[b] = t2
            nc.vector.tensor_copy(t2[0:C, :], tr[0:C, :])
            nc.sync.dma_start(out=t2[C:128, :], in_=t2[0:C, :])
            h2 = xpool.tile([128, TV], f16, name="ha2", tag="ha2")
            ha2[b] = h2
            for (h0, h1) in HALVES:
                hps = psA.tile([128, 800], f32, name="hps", tag="a")
                for (c0, c1) in CH_H:
                    nc.tensor.matmul(hps[:, c0:c1], lhsT=gr2,
                                     rhs=tr[0:C, h0 + c0:h0 + c1],
                                     start=True, stop=True)
                nc.vector.tensor_copy(h2[:, h0:h1], hps)
            vt1[b] = xpool.tile([128, NSB, IC + 1], f16, name="vt1", tag="vt1")
            eb[b] = xpool.tile([128, NSB, TV], f16, name="eb", tag="eb")
            p2[b] = xpool.tile([C, TV], f32, name="p2", tag="p2")
            rxc[b] = xpool.tile([128, NSB], f32, name="rxc", tag="rxc")
            vps = psA.tile([128, NSB, IC + 2], f32, name="vps", tag="a")
            for j, (off, p) in enumerate(SB):
                nc.tensor.matmul(vps[0:p, j, :], lhsT=tr[:, off:off + p],
                                 rhs=wvt_r, start=True, stop=True)
            nc.vector.tensor_copy(vt1[b], vps[:, :, 0:IC + 1])
            # rX per s-block, transposed into partitions via tiny matmuls
            rxp = psA.tile([128, NSB, 2], f32, name="rxp", tag="a")
            for j, (off, p) in enumerate(SB):
                nc.tensor.matmul(rxp[0:p, j, :], lhsT=t2[0:C, off:off + p],
                                 rhs=rv16, start=True, stop=True)
            nc.vector.tensor_scalar_add(rxc[b], rxp[:, :, 0], EXPB)

        def phase1(b):
            """Row-packed A^T pairs -> exp -> col-packed PV pairs.
            PV issue lags A by LAG blocks so a PSUM-buffer wait on the
            (in-order) PE queue never starves the exp pipeline."""
            pacc = psB.tile([128, 800], f32, name="pacc", tag="b")
            nc.tensor.matmul(pacc[:, 0:512], lhsT=zcol, rhs=zrhs[:, 0:512],
                             start=True, stop=False, skip_group_check=True)
            nc.tensor.matmul(pacc[:, 512:800], lhsT=zcol, rhs=zrhs[:, 0:288],
                             start=True, stop=False, skip_group_check=True)
            LAG = 3
            for jj in range(NSB + LAG):
                if jj < NSB:
                    j, (off, p) = jj, SB[jj]
                    aps = [psA.tile([128, 800], f32, name=f"aps{hi}", tag="a")
                           for hi in range(2)]
                    for (c0, c1) in CH_H:
                        for hi, (h0, h1) in enumerate(HALVES):
                            nc.tensor.matmul(
                                aps[hi][0:p, c0:c1],
                                lhsT=ha2[b][C * hi:C * hi + C, off:off + p],
                                rhs=xr2[b][C * hi:C * hi + C, h0 + c0:h0 + c1],
                                start=True, stop=True)
                    for hi, (h0, h1) in enumerate(HALVES):
                        nc.scalar.activation(eb[b][0:p, j, h0:h1],
                                             aps[hi][0:p, :], AF.Exp,
                                             bias=rxc[b][0:p, j:j + 1])
                if jj >= LAG:
                    j, (off, p) = jj - LAG, SB[jj - LAG]
                    for (c0, c1) in CH_H:
                        for ti, (t0, t1) in enumerate(HALVES):
                            nc.tensor.matmul(
                                pacc[C * ti:C * ti + IC + 1, c0:c1],
                                lhsT=vt1[b][0:p, j, :],
                                rhs=eb[b][0:p, j, t0 + c0:t0 + c1],
                                start=False, stop=(j == NSB - 1),
                                skip_group_check=True)
            return pacc

        def remainder(b, pacc):
            """PD -> Wt conv -> /D -> bn_stats per chunk (f16 matmuls,
            row strips 0-63 / 64-127 hold the two t-halves)."""
            pd = workp.tile([128, 800], f16, name="pd", tag="pd")
            nc.vector.tensor_copy(pd, pacc)
            for ti, (t0, t1) in enumerate(HALVES):
                o = C * ti
                for ci, (c0, c1) in enumerate(CH_H):
                    w = c1 - c0
                    p2ps = psB.tile([C, 512], f32, name="p2ps", tag="b")
                    dps = psB.tile([C, 512], f32, name="dps", tag="b")
                    nc.tensor.matmul(p2ps[:, 0:w], lhsT=wt128[o:o + IC, :],
                                     rhs=pd[o:o + IC, c0:c1],
                                     start=True, stop=True)
                    nc.tensor.matmul(dps[:, 0:w],
                                     lhsT=ones128[o + IC:o + IC + 1, :],
                                     rhs=pd[o + IC:o + IC + 1, c0:c1],
                                     start=True, stop=True,
                                     tile_position=(o + IC, 0))
                    rrep = workp.tile([C, 512], f32, name="rrep", tag="rrep")
                    nc.vector.reciprocal_approx_fast(out=rrep[:, 0:w],
                                                     in_=dps[:, 0:w])
                    nc.vector.tensor_mul(p2[b][:, t0 + c0:t0 + c1],
                                         p2ps[:, 0:w], rrep[:, 0:w])
                    nc.vector.bn_stats(stats[:, 4 * b + 2 * ti + ci, :],
                                       p2[b][:, t0 + c0:t0 + c1])

        # prologues for BOTH batches first: keeps phase1(0)+phase1(1)
        # back-to-back with no PE/ACT drain at the batch boundary.
        prologue(0)
        prologue(1)
        pa0 = phase1(0)

        # ---------------- channel gate (PE/DVE slack under phase1) ------
        for b in range(NB):
            nc.vector.reduce_sum(avgs[:, b:b + 1], xa[b][0:C, :], axis=AX.X)
        hps2 = psA.tile([R, NB], f32, name="hps2", tag="a")
        nc.tensor.matmul(hps2, lhsT=w1t, rhs=avgs, start=True, stop=True)
        h_pre = statp.tile([R, NB], f32)
        nc.vector.tensor_scalar(h_pre, hps2, 1.0 / TV, b1_sb,
                                op0=ALU.mult, op1=ALU.add)
        h_sb = statp.tile([R, NB], f32)
        nc.vector.tensor_scalar_max(h_sb, h_pre, 0.0)
        zps = psA.tile([C, NB], f32, name="zps", tag="a")
        nc.tensor.matmul(zps, lhsT=w2t, rhs=h_sb, start=True, stop=True)
        eg = statp.tile([C, NB], f32)
        nc.scalar.activation(eg, zps, AF.Exp, bias=b2n, scale=-1.0)
        gp1 = statp.tile([C, NB], f32)
        nc.vector.tensor_scalar_add(gp1, eg, 1.0)
        gate = statp.tile([C, NB], f32)
        nc.vector.reciprocal(gate, gp1)

        remainder(0, pa0)
        pa1 = phase1(1)
        remainder(1, pa1)

        # ---------------- BN stats: local -> allreduce -> global --------
        mv = statp.tile([C, 2], f32)
        nc.vector.bn_aggr(out=mv, in_=stats)
        m2 = statp.tile([C, 1], f32)
        nc.vector.tensor_mul(m2, mv[:, 0:1], mv[:, 0:1])
        ex2 = statp.tile([C, 1], f32)
        nc.vector.tensor_add(ex2, mv[:, 1:2], m2)
        sums = statp.tile([C, 2], f32)
        cnt_local = float(NB * TV)
        nc.vector.tensor_scalar_mul(sums[:, 0:1], mv[:, 0:1], cnt_local)
        nc.vector.tensor_scalar_mul(sums[:, 1:2], ex2, cnt_local)

        cc_in = dramp.tile([C, 2], f32, name="cc_in")
        cc_out = dramp.tile([C, 2], f32, name="cc_out", addr_space="Shared")
        nc.sync.dma_start(out=cc_in, in_=sums)
        nc.gpsimd.collective_compute(
            "AllReduce",
            ALU.add,
            ins=[cc_in.opt()],
            outs=[cc_out.opt()],
            replica_groups=[list(range(N_CORES))],
        )

        # w_b = gate (.) p2_b overlaps the collective
        wts = [None] * NB
        for b in range(NB):
            u = workp.tile([C, TV], f32, name="u", tag="u")
            wts[b] = u
            nc.vector.tensor_scalar_mul(u, p2[b], gate[:, b:b + 1])

        gs = statp.tile([C, 2], f32)
        nc.sync.dma_start(out=gs, in_=cc_out)

        inv_cnt = 1.0 / (N * TV)
        mean_g = statp.tile([C, 1], f32)
        nc.vector.tensor_scalar_mul(mean_g, gs[:, 0:1], inv_cnt)
        q_g = statp.tile([C, 1], f32)
        nc.vector.tensor_scalar_mul(q_g, gs[:, 1:2], inv_cnt)
        mg2 = statp.tile([C, 1], f32)
        nc.vector.tensor_mul(mg2, mean_g, mean_g)
        var_g = statp.tile([C, 1], f32)
        nc.vector.tensor_sub(var_g, q_g, mg2)
        ve = statp.tile([C, 1], f32)
        nc.vector.tensor_scalar_add(ve, var_g, EPS)
        magic = statp.tile([C, 1], mybir.dt.int32)
        nc.vector.memset(magic, 0x5F3759DF)
        hsh = statp.tile([C, 1], mybir.dt.int32)
        nc.vector.tensor_scalar(hsh, ve.bitcast(mybir.dt.int32), 1, None,
                                op0=ALU.arith_shift_right)
        yi = statp.tile([C, 1], mybir.dt.int32)
        nc.vector.tensor_sub(yi, magic, hsh)
        r1 = statp.tile([C, 1], f32)
        rstd = statp.tile([C, 1], f32)
        t1 = statp.tile([C, 1], f32)
        t3 = statp.tile([C, 1], f32)
        y = yi.bitcast(f32)
        for it, dst in ((0, r1), (1, rstd)):
            nc.vector.tensor_mul(t1, y, y)
            nc.vector.tensor_mul(t1, t1, ve)
            nc.vector.tensor_scalar(t3, t1, -0.5, 1.5, op0=ALU.mult, op1=ALU.add)
            nc.vector.tensor_mul(dst, y, t3)
            y = dst
        sc = statp.tile([C, 1], f32)
        nc.vector.tensor_mul(sc, gamma_sb, rstd)
        msc = statp.tile([C, 1], f32)
        nc.vector.tensor_mul(msc, mean_g, sc)
        sh = statp.tile([C, 1], f32)
        nc.vector.tensor_sub(sh, beta_sb, msc)

        # ------------- finalize: out = sc*(gate*p2) + (x + gate*sh) -----
        for b in range(NB):
            d_b = statp.tile([C, 1], f32, name=f"d_{b}")
            nc.vector.tensor_mul(d_b, gate[:, b:b + 1], sh)
            x3 = workp.tile([C, TV], f32, name="x3", tag="x3")
            nc.scalar.activation(x3, xa[b][0:C, :], AF.Identity, bias=d_b)
            osb = workp.tile([C, TV], f32, name="osb", tag="osb")
            nc.vector.scalar_tensor_tensor(out=osb, in0=wts[b], scalar=sc,
                                           in1=x3, op0=ALU.mult, op1=ALU.add)
            nc.sync.dma_start(out=out_d[b], in_=osb)


_CACHE = {}


def _get_compiled():
    if "nc" in _CACHE:
        return _CACHE["nc"]
    import concourse.bacc as bacc

    nc = bacc.Bacc("TRN2", target_bir_lowering=False, debug=False,
                   enable_asserts=False, num_devices=N_CORES)
    _build(nc)
    nc.compile()
    _CACHE["nc"] = nc
    return nc


def _run(inputs, trace=False, **kw):
    from concourse import bass_utils

    nc = _get_compiled()
    x = np.ascontiguousarray(np.asarray(inputs["x"], dtype=np.float32))
    x = x.reshape(N, C, TV)
    f = lambda a: np.ascontiguousarray(np.asarray(a, dtype=np.float32))
    common = {
        "wq": f(inputs["Wq"]),
        "wk": f(inputs["Wk"]),
        "bk": f(inputs["bk"]).reshape(IC, 1),
        "wv": np.ascontiguousarray(f(inputs["Wv"]).T),
        "bv": f(inputs["bv"]).reshape(1, IC),
        "wt": np.ascontiguousarray(f(inputs["Wt"]).T),
        "bt": f(inputs["bt"]).reshape(1, C),
        "gamma": f(inputs["gamma"]).reshape(C, 1),
        "beta": f(inputs["beta"]).reshape(C, 1),
        "w1": np.ascontiguousarray(f(inputs["W1"]).T),
        "b1": f(inputs["b1"]).reshape(C // 16, 1),
        "w2": np.ascontiguousarray(f(inputs["W2"]).T),
        "b2": f(inputs["b2"]).reshape(C, 1),
    }
    in_maps = []
    for c in range(N_CORES):
        m = dict(common)
        m["x_in"] = np.ascontiguousarray(x[c * NB:(c + 1) * NB])
        in_maps.append(m)
    try:
        res = bass_utils.run_bass_kernel_spmd(
            nc, in_maps, core_ids=list(range(N_CORES)), trace=trace, **kw)
    except Exception:
        import time as _time
        _time.sleep(5)
        res = bass_utils.run_bass_kernel_spmd(
            nc, in_maps, core_ids=list(range(N_CORES)), trace=False, **kw)
    out = np.concatenate([res.results[c]["out"] for c in range(N_CORES)], axis=0)
    return out.reshape(N, C, T, V).astype(np.float32), res


def kernel(**inputs):
    return _run(inputs, trace=False)[0]
